# revision 7
# baseline (speedup 1.0000x reference)
"""Trainium2 Bass kernel for the Attention3 module (B=128, S=1024, RNN=2048, HID=512).

Strategy: data-parallel over batch B across 8 NeuronCores (16 batches/core).
Host side only reshapes/transposes inputs into DMA-friendly layouts; all model
compute (MLP, tanh, scores, softmax, weighted sum) runs on device.

Per-core device pipeline:
  1. MLP: att_h = h@W1.T+b1 @W2.T+b2 @W3.T+b3 @W4.T+b4   (PE, fp32)
     - activations kept transposed ([K,16] lhsT tiles); weights pre-transposed
       on host so the contraction dim lands on partitions.
     - biases folded in as K=1 ones-outer-product matmuls into the same PSUM
       accumulation group.
  2. scores: tanh(p_att^T + att_h) with HID on partitions, so the att_h add is
     a fused per-partition bias on ScalarE; Wa contraction is a PE matmul with
     a 16-column-replicated Wa as lhsT (all output rows identical => row b is
     partition-aligned with scores[b]).  Mask+ba applied as a precomputed
     additive term during PSUM evacuation.
  3. softmax over S on [16, 1024] (free-dim reductions + fused exp bias).
  4. weighted sum: PE-transpose softmax weights to [S, b] column layout, then
     stream att_feats tiles [128, 2048] and matmul (float32r) with the weight
     column as lhsT; row b of each [16, 512] PSUM is the real output.
"""

import functools

import numpy as np

import concourse.bacc as bacc
import concourse.tile as tile
from concourse import mybir
from concourse.bass_utils import run_bass_kernel_spmd
from concourse.masks import make_identity

N_CORES = 8
B, S, RNN, HID = 128, 1024, 2048, 512
BPC = B // N_CORES  # batches per core
F32 = mybir.dt.float32
F32R = mybir.dt.float32r
MASK_NEG = -1.0e9
AX_X = mybir.AxisListType.X
TANH = mybir.ActivationFunctionType.Tanh
EXP = mybir.ActivationFunctionType.Exp


def _build_body(ctx, tc, io):
    nc = tc.nc

    consts = ctx.enter_context(tc.tile_pool(name="consts", bufs=1))
    wpool = ctx.enter_context(tc.tile_pool(name="wpool", bufs=4))
    mlp = ctx.enter_context(tc.tile_pool(name="mlp", bufs=1))
    ppool = ctx.enter_context(tc.tile_pool(name="ppool", bufs=5))
    thpool = ctx.enter_context(tc.tile_pool(name="thpool", bufs=5))
    fpool = ctx.enter_context(tc.tile_pool(name="fpool", bufs=8))
    psA = ctx.enter_context(tc.tile_pool(name="psA", bufs=3, space="PSUM"))
    psB = ctx.enter_context(tc.tile_pool(name="psB", bufs=4, space="PSUM"))

    # ---- constants / small inputs ----
    ident = consts.tile([128, 128], F32)
    make_identity(nc, ident)
    ones1 = consts.tile([1, BPC], F32)
    nc.vector.memset(ones1, 1.0)

    bias_sb = []
    for i, o in enumerate([1024, 1024, 512, 512]):
        t = consts.tile([1, o], F32, tag=f"b{i + 1}")
        nc.sync.dma_start(out=t, in_=io[f"b{i + 1}"])
        bias_sb.append(t)

    wa_sb = consts.tile([128, (HID // 128) * BPC * BPC], F32R)
    nc.sync.dma_start(out=wa_sb, in_=io["warep"])
    wa_m = wa_sb.rearrange("p (t b m) -> p t b m", t=HID // 128, b=BPC)

    madd_sb = consts.tile([BPC, S], F32)
    nc.sync.dma_start(out=madd_sb, in_=io["madd"])

    hT_sb = consts.tile([128, RNN // 128, BPC], F32)
    for k in range(RNN // 128):
        nc.sync.dma_start(out=hT_sb[:, k, :], in_=io["hT"][k * 128 : (k + 1) * 128, :])

    # ---- phase 1: MLP ----
    def layer(xT_sb, K, O, wt_dram, bias_t, name):
        y_sb = mlp.tile([BPC, O], F32, tag=f"y_{name}")
        nch = O // 512
        pss = [
            psA.tile([BPC, 512], F32, tag="ps_small", name=f"ps_y{name}_{n}")
            for n in range(nch)
        ]
        for n in range(nch):
            nc.tensor.matmul(
                pss[n],
                lhsT=ones1,
                rhs=bias_t[0:1, n * 512 : (n + 1) * 512],
                start=True,
                stop=False,
            )
        kt = K // 128
        for k in range(kt):
            wt = wpool.tile([128, O], F32, tag="wt")
            nc.sync.dma_start(out=wt, in_=wt_dram[k * 128 : (k + 1) * 128, :])
            for n in range(nch):
                nc.tensor.matmul(
                    pss[n],
                    lhsT=xT_sb[:, k, :],
                    rhs=wt[:, n * 512 : (n + 1) * 512],
                    start=False,
                    stop=(k == kt - 1),
                )
        for n in range(nch):
            nc.scalar.copy(out=y_sb[:, n * 512 : (n + 1) * 512], in_=pss[n])
        return y_sb

    def transpose_rows(y_sb, O, name):
        yT = mlp.tile([128, O // 128, BPC], F32, tag=f"yT_{name}")
        for j in range(O // 128):
            ps = psA.tile([128, BPC], F32, tag="ps_small")
            nc.tensor.transpose(ps, y_sb[:, j * 128 : (j + 1) * 128], ident[:BPC, :BPC])
            nc.vector.tensor_copy(out=yT[:, j, :], in_=ps)
        return yT

    y1 = layer(hT_sb, RNN, 1024, io["w1t"], bias_sb[0], "1")
    y1T = transpose_rows(y1, 1024, "1")
    y2 = layer(y1T, 1024, 1024, io["w2t"], bias_sb[1], "2")
    y2T = transpose_rows(y2, 1024, "2")
    y3 = layer(y2T, 1024, 512, io["w3t"], bias_sb[2], "3")
    y3T = transpose_rows(y3, 512, "3")
    ah = layer(y3T, 512, 512, io["w4t"], bias_sb[3], "4")
    ahT = transpose_rows(ah, 512, "ah")  # [128, HID//128, BPC]

    # ---- phase 2: scores = Wa . tanh(p^T + att_h) + (mask_add + ba) ----
    # lhsT column m of wa_m[:, ht, b, :] is Wa if m == b else 0, so batch b's
    # matmuls only contribute to PSUM row b; all 16 batches (x nht k-tiles)
    # accumulate into one [BPC, 512] PSUM group per s-half.
    scores = mlp.tile([BPC, S], F32, tag="scores")
    nht = HID // 128
    nsh = S // 512
    ps_sc = [
        psA.tile([BPC, 512], F32, tag="ps_small", name=f"ps_sc_{sh}")
        for sh in range(nsh)
    ]
    for b in range(BPC):
        th_tiles = []
        for ht in range(nht):
            pt = ppool.tile([128, S], F32, tag="pt")
            nc.sync.dma_start(out=pt, in_=io["pT"][b, ht * 128 : (ht + 1) * 128, :])
            th = thpool.tile([128, S], F32R, tag="th")
            nc.scalar.activation(
                out=th, in_=pt, func=TANH, bias=ahT[:, ht, b : b + 1], scale=1.0
            )
            th_tiles.append(th)
        for sh in range(nsh):
            for ht in range(nht):
                nc.tensor.matmul(
                    ps_sc[sh],
                    lhsT=wa_m[:, ht, b, :],
                    rhs=th_tiles[ht][:, sh * 512 : (sh + 1) * 512],
                    start=(b == 0 and ht == 0),
                    stop=(b == BPC - 1 and ht == nht - 1),
                )
    for sh in range(nsh):
        nc.vector.tensor_add(
            out=scores[:, sh * 512 : (sh + 1) * 512],
            in0=ps_sc[sh],
            in1=madd_sb[:, sh * 512 : (sh + 1) * 512],
        )

    # ---- phase 2.5: softmax over S + transpose weights to [S, b] columns ----
    mx = mlp.tile([BPC, 1], F32, tag="mx")
    nc.vector.reduce_max(out=mx, in_=scores, axis=AX_X)
    nmx = mlp.tile([BPC, 1], F32, tag="nmx")
    nc.vector.tensor_scalar_mul(out=nmx, in0=mx, scalar1=-1.0)
    wexp = mlp.tile([BPC, S], F32, tag="wexp")
    ssum = mlp.tile([BPC, 1], F32, tag="ssum")
    nc.scalar.activation(
        out=wexp, in_=scores, func=EXP, bias=nmx, scale=1.0, accum_out=ssum
    )
    rs = mlp.tile([BPC, 1], F32, tag="rs")
    nc.vector.reciprocal(out=rs, in_=ssum)
    wn = mlp.tile([BPC, S], F32, tag="wn")
    nc.vector.tensor_scalar_mul(out=wn, in0=wexp, scalar1=rs)

    # Block-diagonal masked weights: w_mask[:, t, b, m] = w[s, b] if m == b
    # else 0, so batch b's matvec only writes PSUM row b.  Zeroed via a cast
    # copy from an f32 scratch (memset can't encode float32r), then the
    # PE-transposed softmax weights are written straight onto the diagonal.
    import concourse.bass as bass

    nst = S // 128
    w_mask = mlp.tile([128, nst, BPC, BPC], F32R, tag="w_mask")
    zsrc = mlp.tile([128, nst * BPC * BPC], F32, tag="zsrc")
    nc.vector.memset(zsrc, 0.0)
    nc.vector.tensor_copy(
        out=w_mask.rearrange("p a b c -> p (a b c)"), in_=zsrc
    )
    for t in range(nst):
        ps = psA.tile([128, BPC], F32, tag="ps_small")
        nc.tensor.transpose(ps, wn[:, t * 128 : (t + 1) * 128], ident[:BPC, :BPC])
        sl = w_mask[:, t, :, :]
        diag_ap = bass.AP(
            tensor=sl.tensor,
            offset=sl.offset,
            ap=[sl.ap[0], [sl.ap[1][0] + sl.ap[2][0], BPC]],
        )
        nc.vector.tensor_copy(out=diag_ap, in_=ps)

    # ---- phase 3: att_res[b] = sum_s w[b,s] * att_feats[b,s,:] ----
    out_sb = mlp.tile([BPC, RNN], F32, tag="out_sb")
    nn = RNN // 512
    ps_mv = [
        psB.tile([BPC, 512], F32, tag="mv", name=f"ps_mv_{n}") for n in range(nn)
    ]
    for b in range(BPC):
        for t in range(nst):
            ft = fpool.tile([128, RNN], F32R, tag="ft")
            nc.sync.dma_start(out=ft, in_=io["f"][b, t * 128 : (t + 1) * 128, :])
            for n in range(nn):
                nc.tensor.matmul(
                    ps_mv[n],
                    lhsT=w_mask[:, t, b, :],
                    rhs=ft[:, n * 512 : (n + 1) * 512],
                    start=(b == 0 and t == 0),
                    stop=(b == BPC - 1 and t == nst - 1),
                )
    for n in range(nn):
        nc.vector.tensor_copy(
            out=out_sb[:, n * 512 : (n + 1) * 512], in_=ps_mv[n]
        )
    nc.sync.dma_start(out=io["out"], in_=out_sb)


def _build():
    from contextlib import ExitStack

    nc = bacc.Bacc("TRN2", target_bir_lowering=False, debug=False, num_devices=N_CORES)
    io = {
        "hT": nc.dram_tensor("hT", [RNN, BPC], F32, kind="ExternalInput").ap(),
        "pT": nc.dram_tensor("pT", [BPC, HID, S], F32, kind="ExternalInput").ap(),
        "f": nc.dram_tensor("f", [BPC, S, RNN], F32R, kind="ExternalInput").ap(),
        "madd": nc.dram_tensor("madd", [BPC, S], F32, kind="ExternalInput").ap(),
        "w1t": nc.dram_tensor("w1t", [RNN, 1024], F32, kind="ExternalInput").ap(),
        "w2t": nc.dram_tensor("w2t", [1024, 1024], F32, kind="ExternalInput").ap(),
        "w3t": nc.dram_tensor("w3t", [1024, 512], F32, kind="ExternalInput").ap(),
        "w4t": nc.dram_tensor("w4t", [512, 512], F32, kind="ExternalInput").ap(),
        "b1": nc.dram_tensor("b1", [1, 1024], F32, kind="ExternalInput").ap(),
        "b2": nc.dram_tensor("b2", [1, 1024], F32, kind="ExternalInput").ap(),
        "b3": nc.dram_tensor("b3", [1, 512], F32, kind="ExternalInput").ap(),
        "b4": nc.dram_tensor("b4", [1, 512], F32, kind="ExternalInput").ap(),
        "warep": nc.dram_tensor(
            "warep", [128, (HID // 128) * BPC * BPC], F32R, kind="ExternalInput"
        ).ap(),
        "out": nc.dram_tensor("out", [BPC, RNN], F32, kind="ExternalOutput").ap(),
    }
    with tile.TileContext(nc) as tc:
        with ExitStack() as ctx:
            _build_body(ctx, tc, io)
    nc.compile()
    return nc


@functools.lru_cache(maxsize=1)
def _get_nc():
    return _build()


def _prep_in_maps(h, att_feats, p_att_feats, mask, W1, b1, W2, b2, W3, b3, W4, b4, Wa, ba):
    f32 = np.float32
    asc = np.ascontiguousarray

    def a(x):
        return np.asarray(x, dtype=f32)

    w1t = asc(a(W1).T)
    w2t = asc(a(W2).T)
    w3t = asc(a(W3).T)
    w4t = asc(a(W4).T)
    b1r = a(b1).reshape(1, -1)
    b2r = a(b2).reshape(1, -1)
    b3r = a(b3).reshape(1, -1)
    b4r = a(b4).reshape(1, -1)
    wa = a(Wa).reshape(-1)  # [HID]
    warep = np.zeros((128, HID // 128, BPC, BPC), dtype=f32)
    for ht in range(HID // 128):
        for b in range(BPC):
            warep[:, ht, b, b] = wa[ht * 128 : (ht + 1) * 128]
    warep = warep.reshape(128, (HID // 128) * BPC * BPC)
    ba0 = float(np.asarray(ba).reshape(-1)[0])

    h = a(h)
    p = a(p_att_feats)
    f = np.asarray(att_feats, dtype=f32)
    m = np.asarray(mask)

    in_maps = []
    for c in range(N_CORES):
        sl = slice(c * BPC, (c + 1) * BPC)
        in_maps.append(
            {
                "hT": asc(h[sl].T),
                "pT": asc(p[sl].transpose(0, 2, 1)),
                "f": asc(f[sl]),
                "madd": (m[sl].astype(f32) * MASK_NEG + ba0).astype(f32),
                "w1t": w1t,
                "w2t": w2t,
                "w3t": w3t,
                "w4t": w4t,
                "b1": b1r,
                "b2": b2r,
                "b3": b3r,
                "b4": b4r,
                "warep": warep,
            }
        )
    return in_maps


def _run(in_maps, trace=False):
    nc = _get_nc()
    res = run_bass_kernel_spmd(nc, in_maps, core_ids=list(range(N_CORES)), trace=trace)
    out = np.concatenate([res.results[c]["out"] for c in range(N_CORES)], axis=0)
    return out, res


def kernel(h, att_feats, p_att_feats, mask, W1, b1, W2, b2, W3, b3, W4, b4, Wa, ba):
    in_maps = _prep_in_maps(
        h, att_feats, p_att_feats, mask, W1, b1, W2, b2, W3, b3, W4, b4, Wa, ba
    )
    out, _ = _run(in_maps)
    return out
